# revision 1
# baseline (speedup 1.0000x reference)
"""2-layer GAT (graph attention) on 8 Trainium2 NeuronCores.

Sharding: (head x row-half). Core c owns head c%4 and query rows
[(c//4)*2048 : +2048). Each core computes h = x @ W1[head] for ALL 4096 keys
locally (TensorE is cheap: ~14us), which eliminates the large h AllGather a
row-sharded layout would need (~8.5MB/core at ~62GB/s wire). The cross-head
mean becomes two 4-rank ReduceScatters on bf16 [1024, 256] partials, the
first overlapping the second attention half. Scores use the rank-1 identity
exp(leakyrelu(es+ed)) = max(exp(es)exp(ed), exp(a*es)exp(a*ed)): even key
blocks ride ScalarE (Prelu+Exp with fused per-partition bias), odd blocks
are pure-DVE (tensor_scalar 4x mode + scalar_tensor_tensor), balancing the
two engines at ~34us each per attention half. The edge mask multiplies
after exp (identical to -inf masking); softmax denominators come free via a
ones column on the value matrix. e_dst rides the h matmul as a 257th output
column. Layer 2 gathers a packed [512, 18] payload (h2 | ones | e2_dst)
over all 8 cores and ends with a batched log-softmax. Layer-2 rows per core
are the ReduceScatter shards; the host permutes rows back via
out_rows_for_core, and pre-permutes mask2T to match the AllGather key
order.
"""

import os

import numpy as np
import ml_dtypes

_STOP = os.environ.get("K_STOP", "")       # bench-only: B, D0, D, RS, E
_DO_RS = os.environ.get("K_RS", "1") == "1"
_RED = os.environ.get("K_RED", "rs2")      # rs2 | rs1 | rs22 | rs8
RS_PAIRS1 = [[0, 1], [2, 3], [4, 5], [6, 7]]
RS_PAIRS2 = [[0, 2], [1, 3], [4, 6], [5, 7]]

import concourse.bass as bass
import concourse.tile as tile
from concourse import bacc, mybir
from concourse.bass_utils import run_bass_kernel_spmd
from concourse.masks import make_identity

P = 128
N, F, O, H, C = 4096, 512, 256, 4, 16
NCORES = 8
QL = N // 2              # 2048 query rows per core (layer 1)
QH = QL // 2             # 1024-row attention half
Q2 = N // NCORES         # 512 layer-2 rows per core
MB = N // P              # 32 key blocks
KB = F // P              # 4 contraction blocks over F
OB = O // P              # 2 contraction blocks over O
ALPHA = 0.2
PAY = C + 2              # payload cols: 0:16 h2, 16 ones, 17 e2_dst

bf16 = mybir.dt.bfloat16
f32 = mybir.dt.float32
f8 = mybir.dt.float8e4
_RDT = f8 if _RED == "rs8" else bf16
AF = mybir.ActivationFunctionType
ALU = mybir.AluOpType
AX = mybir.AxisListType

RS_GROUPS = [[0, 1, 2, 3], [4, 5, 6, 7]]


def _build(reps=1):
    nc = bacc.Bacc("TRN2", target_bir_lowering=False, debug=False,
                   num_devices=NCORES)

    xT_d = nc.dram_tensor("xT", [F, N], bf16, kind="ExternalInput").ap()
    xTq_d = nc.dram_tensor("xTq", [F, QL], bf16, kind="ExternalInput").ap()
    w1e_d = nc.dram_tensor("w1e", [F, O + 1], bf16, kind="ExternalInput").ap()
    wsrc_d = nc.dram_tensor("wsrc", [F, 1], bf16, kind="ExternalInput").ap()
    maskT_d = nc.dram_tensor("maskT", [N, QL], bf16, kind="ExternalInput").ap()
    mask2T_d = nc.dram_tensor("mask2T", [N, Q2], bf16,
                              kind="ExternalInput").ap()
    w2p_d = nc.dram_tensor("w2p", [O, PAY], bf16, kind="ExternalInput").ap()
    out_d = nc.dram_tensor("out", [Q2, C], f32, kind="ExternalOutput").ap()

    with tile.TileContext(nc) as tc:
        for _ in range(reps):
            _emit(tc, xT_d, xTq_d, w1e_d, wsrc_d, maskT_d, mask2T_d, w2p_d,
                  out_d)
    nc.compile()
    return nc


def _emit(tc, xT_d, xTq_d, w1e_d, wsrc_d, maskT_d, mask2T_d, w2p_d, out_d):
    nc = tc.nc
    with tc.tile_pool(name="singles", bufs=1) as singles:
        # ---- persistent SBUF tensors ----
        xT_sb = singles.tile([P, KB, N], bf16)
        xTq_sb = singles.tile([P, KB, QL], bf16)
        w1e_sb = singles.tile([P, KB, O + 1], bf16)
        wsrc_sb = singles.tile([P, KB, 1], bf16)
        w2p_sb = singles.tile([P, OB, PAY], bf16)
        ones1 = singles.tile([1, P], bf16)
        ident = singles.tile([P, P], bf16)
        h_sb = singles.tile([P, MB, O + 1], bf16)   # h | ones col
        edst = singles.tile([P, MB], f32)
        Ek = singles.tile([P, MB], f32)
        Fk = singles.tile([P, MB], f32)
        esb = singles.tile([P, QL], f32)
        Eqb = singles.tile([P, QL], bf16)
        Fqb = singles.tile([P, QL], bf16)
        esrow = singles.tile([1, QL], bf16)
        x2p = singles.tile([P, 16, O], _RDT)
        x2raw = singles.tile([P, 4, O], _RDT)
        x2_sb = singles.tile([P, 4, O], bf16)
        x2T = singles.tile([P, OB, Q2], bf16)
        pay_sb = singles.tile([P, 4, PAY], bf16)
        h2g_sb = singles.tile([P, MB, PAY], bf16)
        e2d_all = singles.tile([P, MB], f32)
        Ek2 = singles.tile([P, MB], f32)
        Fk2 = singles.tile([P, MB], f32)
        e2b = singles.tile([P, Q2], f32)
        Eq2b = singles.tile([P, Q2], bf16)
        Fq2b = singles.tile([P, Q2], bf16)
        e2row = singles.tile([1, Q2], bf16)
        mask2_sb = singles.tile([P, MB, Q2], bf16)

        # ---- input DMAs: critical-path tensors first (xTq gates phase C,
        # which gates all of D's elementwise; xT only gates h matmuls) ----
        nc.sync.dma_start(wsrc_sb[:],
                          wsrc_d.rearrange("(kb p) c -> p kb c", p=P))
        nc.sync.dma_start(xTq_sb[:], xTq_d.rearrange("(kb p) q -> p kb q", p=P))
        nc.sync.dma_start(w1e_sb[:], w1e_d.rearrange("(kb p) c -> p kb c", p=P))
        nc.sync.dma_start(w2p_sb[:], w2p_d.rearrange("(ob p) c -> p ob c", p=P))
        xT_r = xT_d.rearrange("(kb p) n -> p kb n", p=P)
        for g in range(8):
            s = bass.ts(g, N // 8)
            nc.sync.dma_start(xT_sb[:, :, s], xT_r[:, :, s])
        maskT_r = maskT_d.rearrange("(b p) q -> p b q", p=P)
        mask2T_r = mask2T_d.rearrange("(b p) q -> p b q", p=P)

        for g in range(4):
            s = bass.ts(g, MB // 4)
            nc.sync.dma_start(mask2_sb[:, s, :], mask2T_r[:, s, :])
        nc.vector.memset(ones1[:], 1.0)
        make_identity(nc, ident[:])
        nc.vector.memset(h_sb[:, :, O], float(H))

        # ---- phase C: es row, broadcast, Eq/Fq (concurrent with B) ----
        with tc.tile_pool(name="es_psum", bufs=1, space="PSUM") as epp, \
             tc.tile_pool(name="bc_psum", bufs=1, space="PSUM") as bpp:
            for ch in range(4):
                pse = epp.tile([1, 512], f32, tag="ps_es")
                for kb in range(KB):
                    nc.tensor.matmul(pse[:], wsrc_sb[:, kb, :],
                                     xTq_sb[:, kb, bass.ts(ch, 512)],
                                     start=(kb == 0), stop=(kb == KB - 1))
                nc.vector.tensor_copy(esrow[:, bass.ts(ch, 512)], pse[:])
            psB = bpp.tile([P, QL], f32, tag="psB")
            for ch in range(4):
                nc.tensor.matmul(psB[:, bass.ts(ch, 512)], ones1[:],
                                 esrow[:, bass.ts(ch, 512)],
                                 start=True, stop=True)
            nc.vector.tensor_copy(esb[:], psB[:])
            nc.scalar.activation(Eqb[:], psB[:], AF.Exp)
            nc.scalar.activation(Fqb[:], psB[:], AF.Exp, scale=ALPHA)

            # ---- phase B: h and e_dst for ALL key rows (local, no gather) --
            with tc.tile_pool(name="h_psum", bufs=3, space="PSUM") as hpp:
                for nb in range(MB):
                    ps = hpp.tile([P, O + 1], f32, tag="ps_h")
                    for kb in range(KB):
                        nc.tensor.matmul(ps[:], xT_sb[:, kb, bass.ts(nb, P)],
                                         w1e_sb[:, kb, :],
                                         start=(kb == 0), stop=(kb == KB - 1))
                    if nb % 2 == 0:
                        nc.vector.tensor_copy(h_sb[:, nb, 0:O], ps[:, 0:O])
                    else:
                        nc.scalar.activation(h_sb[:, nb, 0:O], ps[:, 0:O],
                                             AF.Copy)
                    nc.vector.tensor_copy(edst[:, nb:nb + 1], ps[:, O:O + 1])
                # Ek/Fk in block-chunks so attention can start early
                for ch in range(4):
                    s = bass.ts(ch, MB // 4)
                    nc.scalar.activation(Ek[:, s], edst[:, s], AF.Exp)
                    nc.scalar.activation(Fk[:, s], edst[:, s], AF.Exp,
                                         scale=ALPHA)

        if _STOP == "B":
            return
        # ---- phase D: layer-1 attention, two query halves + RS each ----
        x2r_ds = []
        with tc.tile_pool(name="dram1", bufs=1, space="DRAM") as dram1:
          with tc.tile_pool(name="acc_psum", bufs=1, space="PSUM") as accp, \
               tc.tile_pool(name="mask_pool", bufs=8) as mpool, \
               tc.tile_pool(name="pm_pool", bufs=8) as pmp, \
               tc.tile_pool(name="zt_pool", bufs=4) as ztp, \
               tc.tile_pool(name="small1", bufs=4) as sp1:
            for qh in range(2):
                qs = bass.ts(qh, QH)
                accs = [accp.tile([P, O + 1], f32, tag=f"acc{qc}",
                                  name=f"acc{qc}") for qc in range(8)]
                for kb in range(MB):
                    mt = mpool.tile([P, QH], bf16, tag="mt", name="mt")
                    nc.sync.dma_start(mt[:], maskT_r[:, kb, qs])
                    pm = pmp.tile([P, QH], bf16, tag="pm", name="pm")
                    if kb % 2 == 0 or kb == MB - 1:
                        z = ztp.tile([P, QH], bf16, tag="z", name="z")
                        nc.scalar.activation(z[:], esb[:, qs], AF.Prelu,
                                             bias=edst[:, kb:kb + 1],
                                             scale=1.0, alpha=ALPHA)
                        pt = ztp.tile([P, QH], bf16, tag="pt", name="pt")
                        nc.scalar.activation(pt[:], z[:], AF.Exp)
                        nc.vector.tensor_mul(pm[:], pt[:], mt[:])
                    else:
                        t1 = ztp.tile([P, QH], bf16, tag="t1", name="t1")
                        nc.vector.tensor_scalar_mul(t1[:], Eqb[:, qs],
                                                    Ek[:, kb:kb + 1])
                        t2 = ztp.tile([P, QH], bf16, tag="t2", name="t2")
                        nc.vector.scalar_tensor_tensor(
                            t2[:], Fqb[:, qs], Fk[:, kb:kb + 1], t1[:],
                            op0=ALU.mult, op1=ALU.max)
                        nc.vector.tensor_mul(pm[:], t2[:], mt[:])
                    for qc in range(8):
                        nc.tensor.matmul(accs[qc][:], pm[:, bass.ts(qc, P)],
                                         h_sb[:, kb, :],
                                         start=(kb == 0), stop=(kb == MB - 1))
                for qc in range(8):
                    r = sp1.tile([P, 1], f32, tag="r")
                    nc.vector.reciprocal(r[:], accs[qc][:, O:O + 1])
                    nc.vector.tensor_scalar_mul(x2p[:, qh * 8 + qc, :],
                                                accs[qc][:, 0:O], r[:])
                if _RED in ("rs2", "rs8"):
                    x2h_d = dram1.tile([QH, O], _RDT, name=f"x2h{qh}")
                    x2r_d = dram1.tile([QH // 4, O], _RDT, name=f"x2r{qh}")
                    nc.sync.dma_start(
                        x2h_d.rearrange("(b p) c -> p b c", p=P),
                        x2p[:, qh * 8:(qh + 1) * 8, :])
                    if _DO_RS:
                        nc.gpsimd.collective_compute(
                            "ReduceScatter", ALU.add, replica_groups=RS_GROUPS,
                            ins=[x2h_d.opt()], outs=[x2r_d.opt()])
                    x2r_ds.append(x2r_d)
                elif _RED == "rs22":
                    x2h_d = dram1.tile([QH, O], bf16, name=f"x2h{qh}")
                    y_d = dram1.tile([QH // 2, O], bf16, name=f"y{qh}")
                    x2r_d = dram1.tile([QH // 4, O], bf16, name=f"x2r{qh}")
                    nc.sync.dma_start(
                        x2h_d.rearrange("(b p) c -> p b c", p=P),
                        x2p[:, qh * 8:(qh + 1) * 8, :])
                    nc.gpsimd.collective_compute(
                        "ReduceScatter", ALU.add, replica_groups=RS_PAIRS1,
                        ins=[x2h_d.opt()], outs=[y_d.opt()])
                    nc.gpsimd.collective_compute(
                        "ReduceScatter", ALU.add, replica_groups=RS_PAIRS2,
                        ins=[y_d.opt()], outs=[x2r_d.opt()])
                    x2r_ds.append(x2r_d)
                if _STOP == "D0":
                    return
          if _RED == "rs1":
              x2h_d = dram1.tile([QL, O], bf16, name="x2h")
              x2r_d = dram1.tile([Q2, O], bf16, name="x2r")
              nc.sync.dma_start(x2h_d.rearrange("(b p) c -> p b c", p=P),
                                x2p[:])
              if _DO_RS:
                  nc.gpsimd.collective_compute(
                      "ReduceScatter", ALU.add, replica_groups=RS_GROUPS,
                      ins=[x2h_d.opt()], outs=[x2r_d.opt()])
              x2r_ds = [x2r_d]
          if _STOP in ("D", "RS"):
              return
          # ---- phase E: relu, transpose, layer-2 projections ----
          if _RED == "rs1":
              nc.sync.dma_start(x2raw[:],
                                x2r_ds[0].rearrange("(b p) c -> p b c", p=P))
          else:
              for qh in range(2):
                  nc.sync.dma_start(
                      x2raw[:, qh * 2:(qh + 1) * 2, :],
                      x2r_ds[qh].rearrange("(b p) c -> p b c", p=P))
          # per-half relu so half-0 transposes/projections overlap RS1
          nc.vector.tensor_relu(x2_sb[:, 0:2, :], x2raw[:, 0:2, :])
          nc.vector.tensor_relu(x2_sb[:, 2:4, :], x2raw[:, 2:4, :])
          with tc.tile_pool(name="l2_psum", bufs=2, space="PSUM") as lpp:
                for qc in range(4):
                    for ob in range(OB):
                        tp = lpp.tile([P, P], bf16, tag="tp")
                        nc.tensor.transpose(tp[:],
                                            x2_sb[:, qc, bass.ts(ob, P)],
                                            ident[:])
                        nc.vector.tensor_copy(x2T[:, ob, bass.ts(qc, P)],
                                              tp[:])
                for qc in range(4):
                    ps2 = lpp.tile([P, C + 1], f32, tag="ps2")
                    for ob in range(OB):
                        nc.tensor.matmul(ps2[:], x2T[:, ob, bass.ts(qc, P)],
                                         w2p_sb[:, ob, 0:C + 1],
                                         start=(ob == 0), stop=(ob == OB - 1))
                    nc.vector.tensor_copy(pay_sb[:, qc, 0:C], ps2[:, 0:C])
                    nc.vector.tensor_copy(pay_sb[:, qc, C + 1:C + 2],
                                          ps2[:, C:C + 1])
                nc.vector.memset(pay_sb[:, :, C], 1.0)
                ps_e2 = lpp.tile([1, Q2], f32, tag="ps_e2")
                for ob in range(OB):
                    nc.tensor.matmul(ps_e2[:], w2p_sb[:, ob, C + 1:C + 2],
                                     x2T[:, ob, :],
                                     start=(ob == 0), stop=(ob == OB - 1))
                nc.vector.tensor_copy(e2row[:], ps_e2[:])
                psB2 = lpp.tile([P, Q2], f32, tag="psB2")
                nc.tensor.matmul(psB2[:], ones1[:], e2row[:],
                                 start=True, stop=True)
                nc.vector.tensor_copy(e2b[:], psB2[:])
                nc.scalar.activation(Eq2b[:], psB2[:], AF.Exp)
                nc.scalar.activation(Fq2b[:], psB2[:], AF.Exp, scale=ALPHA)

        if _STOP == "E":
            return
        # ---- phase F: AllGather packed payload + layer-2 attention ----
        with tc.tile_pool(name="dram2", bufs=1, space="DRAM") as dram2:
            pay_d = dram2.tile([Q2, PAY], bf16)
            gath_d = dram2.tile([N, PAY], bf16, addr_space="Shared")
            nc.sync.dma_start(pay_d.rearrange("(b p) c -> p b c", p=P),
                              pay_sb[:])
            nc.gpsimd.collective_compute(
                "AllGather", ALU.bypass,
                replica_groups=[list(range(NCORES))],
                ins=[pay_d.opt()], outs=[gath_d.opt()])
            nc.sync.dma_start(h2g_sb[:],
                              gath_d.rearrange("(b p) c -> p b c", p=P))
            nc.vector.tensor_copy(e2d_all[:], h2g_sb[:, :, C + 1])
            nc.scalar.activation(Ek2[:], e2d_all[:], AF.Exp)
            nc.scalar.activation(Fk2[:], e2d_all[:], AF.Exp, scale=ALPHA)

            with tc.tile_pool(name="acc2_psum", bufs=1, space="PSUM") as acc2p, \
                 tc.tile_pool(name="m2_pool", bufs=6) as m2p, \
                 tc.tile_pool(name="pm2_pool", bufs=6) as pm2p, \
                 tc.tile_pool(name="z2_pool", bufs=4) as z2p, \
                 tc.tile_pool(name="small2", bufs=4) as sp2:
                accs2 = [acc2p.tile([P, C + 1], f32, tag=f"a2_{qc}",
                                    name=f"a2_{qc}") for qc in range(4)]
                for kb in range(MB):
                    m2 = mask2_sb[:, kb, :]
                    pm2 = pm2p.tile([P, Q2], bf16, tag="pm2", name="pm2")
                    if kb % 2 == 0:
                        z2 = z2p.tile([P, Q2], bf16, tag="z2", name="z2")
                        nc.scalar.activation(z2[:], e2b[:], AF.Prelu,
                                             bias=e2d_all[:, kb:kb + 1],
                                             scale=1.0, alpha=ALPHA)
                        pt2 = z2p.tile([P, Q2], bf16, tag="pt2", name="pt2")
                        nc.scalar.activation(pt2[:], z2[:], AF.Exp)
                        nc.vector.tensor_mul(pm2[:], pt2[:], m2)
                    else:
                        t1 = z2p.tile([P, Q2], bf16, tag="t12", name="t12")
                        nc.vector.tensor_scalar_mul(t1[:], Eq2b[:],
                                                    Ek2[:, kb:kb + 1])
                        t2 = z2p.tile([P, Q2], bf16, tag="t22", name="t22")
                        nc.vector.scalar_tensor_tensor(
                            t2[:], Fq2b[:], Fk2[:, kb:kb + 1], t1[:],
                            op0=ALU.mult, op1=ALU.max)
                        nc.vector.tensor_mul(pm2[:], t2[:], m2)
                    for qc in range(4):
                        nc.tensor.matmul(accs2[qc][:],
                                         pm2[:, bass.ts(qc, P)],
                                         h2g_sb[:, kb, 0:C + 1],
                                         start=(kb == 0), stop=(kb == MB - 1))
                # log-softmax, batched by activation function
                logits_all = sp2.tile([P, 4, C], f32, tag="logits_all",
                                      name="logits_all")
                negmax_all = sp2.tile([P, 4], f32, tag="negmax_all",
                                      name="negmax_all")
                ssum_all = sp2.tile([P, 4], f32, tag="ssum_all",
                                    name="ssum_all")
                lse_all = sp2.tile([P, 4], f32, tag="lse_all", name="lse_all")
                for qc in range(4):
                    r2 = sp2.tile([P, 1], f32, tag="r2")
                    nc.vector.reciprocal(r2[:], accs2[qc][:, C:C + 1])
                    nc.vector.tensor_scalar_mul(logits_all[:, qc, :],
                                                accs2[qc][:, 0:C], r2[:])
                    nc.vector.reduce_max(negmax_all[:, qc:qc + 1],
                                         logits_all[:, qc, :], axis=AX.X,
                                         negate=True)
                for qc in range(4):
                    expt = sp2.tile([P, C], f32, tag="expt")
                    nc.scalar.activation(expt[:], logits_all[:, qc, :], AF.Exp,
                                         bias=negmax_all[:, qc:qc + 1],
                                         accum_out=ssum_all[:, qc:qc + 1])
                nc.scalar.activation(lse_all[:], ssum_all[:], AF.Ln)
                for qc in range(4):
                    res = sp2.tile([P, C], f32, tag="res")
                    nc.vector.tensor_scalar(res[:], logits_all[:, qc, :],
                                            negmax_all[:, qc:qc + 1],
                                            lse_all[:, qc:qc + 1],
                                            ALU.add, ALU.subtract)
                    nc.sync.dma_start(out_d[bass.ts(qc, P), :], res[:])


def out_rows_for_core(c):
    """Global output row indices handled by core c, in on-device order."""
    qb = (c // 4) * QL
    r = c % 4
    if _RED == "rs1":
        return list(range(qb + 512 * r, qb + 512 * r + 512))
    if _RED == "rs22":
        off = {0: 0, 1: 512, 2: 256, 3: 768}[r]
        rows = list(range(qb + off, qb + off + 256))
        rows += list(range(qb + QH + off, qb + QH + off + 256))
        return rows
    rows = list(range(qb + 256 * r, qb + 256 * r + 256))
    rows += list(range(qb + QH + 256 * r, qb + QH + 256 * r + 256))
    return rows


def prep_in_maps(x, adj, W1, a1, W2, a2):
    bf = ml_dtypes.bfloat16
    x = np.asarray(x, dtype=np.float32)
    adj = np.asarray(adj)
    W1 = np.asarray(W1, dtype=np.float32)
    a1 = np.asarray(a1, dtype=np.float32)
    W2 = np.asarray(W2, dtype=np.float32)
    a2 = np.asarray(a2, dtype=np.float32)

    xT = np.ascontiguousarray(x.T).astype(bf)                     # [F, N]
    wsrc_all = np.einsum("hfo,ho->fh", W1, a1[:, :O])             # [F, H]
    wdst_all = np.einsum("hfo,ho->fh", W1, a1[:, O:])             # [F, H]
    w2p = np.zeros((O, PAY), np.float32)
    w2p[:, 0:C] = W2[0]
    w2p[:, C] = W2[0] @ a2[0, C:]      # e2_dst vector
    w2p[:, C + 1] = W2[0] @ a2[0, :C]  # e2_src vector
    w2p = w2p.astype(bf)
    adj_on = (adj > 0).astype(np.float32)

    rows_all = [out_rows_for_core(c) for c in range(NCORES)]
    perm = [r for rows in rows_all for r in rows]

    in_maps = []
    for c in range(NCORES):
        hd, qb = c % 4, (c // 4) * QL
        w1e = np.concatenate([W1[hd], wdst_all[:, hd:hd + 1]], 1).astype(bf)
        in_maps.append({
            "xT": xT,
            "xTq": np.ascontiguousarray(xT[:, qb:qb + QL]),
            "w1e": w1e,
            "wsrc": np.ascontiguousarray(wsrc_all[:, hd:hd + 1]).astype(bf),
            "maskT": np.ascontiguousarray(adj_on[qb:qb + QL, :].T).astype(bf),
            "mask2T": np.ascontiguousarray(
                adj_on[np.ix_(rows_all[c], perm)].T).astype(bf),
            "w2p": w2p,
        })
    return in_maps


def assemble_out(results):
    out = np.empty((N, C), np.float32)
    for c in range(NCORES):
        out[out_rows_for_core(c)] = results[c]["out"]
    return out


_CACHED = None


def _get_nc():
    global _CACHED
    if _CACHED is None:
        _CACHED = _build()
    return _CACHED


def kernel(x, adj, W1, a1, W2, a2):
    in_maps = prep_in_maps(x, adj, W1, a1, W2, a2)
    nc = _get_nc()
    res = run_bass_kernel_spmd(nc, in_maps, core_ids=list(range(NCORES)))
    return assemble_out(res.results)

